# revision 4
# baseline (speedup 1.0000x reference)
"""CosineHammingAttention on 8 TRN2 NeuronCores.

Host (cheap, control-flow heavy): LSH hashing, stable sort, gathers, power
method, sampling probabilities + jax categorical sampling, masked-weight
variant construction, final unsort/divide.

Device (per (b,h), 2 per core): block-sparse attention over sorted buckets +
importance-sampled residual attention, fused into one accumulated attention
tensor per query group.

Layout notes (per (b,h)):
  - all S = Q.K^T matmuls are computed transposed (S^T = [keys/samples, queries])
    so the exp(S^T) output feeds the A^T @ W matmul directly as the moving
    operand with contraction on partitions.
  - K=64 contractions are row-packed: two independent matmuls run concurrently
    in PE row-groups (0,0)/(64,0), writing different PSUM banks.
  - the (query-block, sample) mask is folded into 16 pre-masked copies of the
    sampled value matrix (exact zeros), so exp needs no per-op bias and m2 is
    a plain matmul.
"""

import os
import sys
import types

import numpy as np
import ml_dtypes

B, N, H, D = 2, 4096, 8, 64
NUM_PROJS, BUCKET, SAMPLE, PM_ITERS = 7, 256, 800, 32
NB = N // BUCKET          # 16 blocks
SPAD = 896                # samples padded to 7*128
NCH = SPAD // 128         # 7 sample chunks
NCORES = 8
BH_PER_CORE = (B * H) // NCORES  # 2

F32 = np.float32
BF16 = ml_dtypes.bfloat16

_NC_CACHE = {}
LAST_RESULT = None  # BassKernelResults of the most recent kernel() call


def _ensure_ntff_hook():
    """Register the axon NTFF profile hook if the image's antenv lacks it."""
    try:
        import antenv.axon_hooks  # noqa: F401
        return
    except ImportError:
        pass
    mod = types.ModuleType("antenv.axon_hooks")
    mod._hook = None
    def _set(h):
        mod._hook = h
    def _get():
        return mod._hook
    mod.set_axon_ntff_profile_hook = _set
    mod.get_axon_ntff_profile_hook = _get
    sys.modules["antenv.axon_hooks"] = mod
    try:
        from trn_agent_boot.trn_boot import _ntff_profile_via_ctypes
        hook = _ntff_profile_via_ctypes("/opt/axon/libaxon_pjrt.so")
        if hook is not None:
            _set(hook)
    except Exception:
        pass


def _hamming_perm(n):
    a = np.array([0, 1], dtype=np.int64)
    for s in range(2, n + 1):
        a = np.concatenate([a, a[::-1] + 2 ** (s - 1)])
    return a


def _host_prep(query, key, value, proj_dir, pm_init):
    """Hash/sort/sample on host. Returns per-(b,h) device inputs + unsort info."""
    import jax
    import jax.numpy as jnp

    cpu = jax.devices("cpu")[0]
    with jax.default_device(cpu):
        q = np.transpose(query, (0, 2, 1, 3)).astype(F32)   # [B,H,N,D]
        k = np.transpose(key, (0, 2, 1, 3)).astype(F32)
        v = np.transpose(value, (0, 2, 1, 3)).astype(F32)

        perm = _hamming_perm(NUM_PROJS)
        pow2 = (2 ** np.arange(NUM_PROJS)).astype(np.int64)

        def lsh(x):
            proj = np.asarray(
                jnp.einsum("bhnd,bhdr->bhnr", jnp.asarray(x), jnp.asarray(proj_dir))
            )
            bits = (proj > 0).astype(np.int64)
            return perm[(bits * pow2).sum(-1)]

        k_idx = np.argsort(lsh(k), axis=2, kind="stable")
        q_idx = np.argsort(lsh(q), axis=2, kind="stable")

        v_aug = np.concatenate([v, np.ones((B, H, N, 1), F32)], axis=-1)
        gat = lambda m, idx: np.take_along_axis(m, idx[..., None], axis=2)
        q_s = gat(q, q_idx)
        k_s = gat(k, k_idx)
        w_s = gat(v_aug, k_idx)

        # power method for the value-Gram spectral norm (matches reference jnp)
        w_j = jnp.asarray(w_s)
        Gram = jnp.einsum("bhnt,bhnd->bhtd", w_j, w_j)
        x = jnp.asarray(pm_init)
        x = x / jnp.linalg.norm(x, axis=2, keepdims=True)
        for _ in range(PM_ITERS):
            y = jnp.einsum("bhnm,bhm->bhn", Gram, x)
            x = y / jnp.linalg.norm(y, axis=2, keepdims=True)
        sigma = jnp.linalg.norm(y, axis=2)

        P = jnp.linalg.norm(w_j, axis=3) / sigma[:, :, None] + 1.0 / N
        P = jnp.maximum(P, 0.0)
        P = P / jnp.sum(P, axis=2, keepdims=True)

        skey = jax.random.key(1234)
        idx = jax.random.categorical(
            skey, jnp.log(P.reshape(B * H, N)), axis=-1, shape=(SAMPLE, B * H)
        ).T
        idx = np.asarray(idx)          # [BH, SAMPLE] sorted-key positions
        P = np.asarray(P)

    per_bh = []
    for bh in range(B * H):
        b, h = bh // H, bh % H
        i = idx[bh]
        blk = i // BUCKET
        order = np.argsort(blk, kind="stable")
        i_s = i[order]
        counts = np.bincount(blk[order], minlength=NB)
        bnd = np.zeros(NB + 1, np.int64)
        bnd[1:] = np.cumsum(counts)

        Kpi = np.zeros((SPAD, D), F32)
        Kpi[:SAMPLE] = k_s[b, h, i_s]
        sig = 1.0 / (P[b, h, i_s] * SAMPLE)
        Wpi = np.zeros((SPAD, D + 1), F32)
        Wpi[:SAMPLE] = sig[:, None] * w_s[b, h, i_s]

        # --- device layouts ---
        qsT = np.empty((128, N), F32)             # duplicated q_s^T (row packing)
        qsT[0:64] = q_s[b, h].T
        qsT[64:128] = qsT[0:64]

        ksT = np.zeros((128, NB * 128), F32)      # packed k_s^T: top=even chunk,
        for g in range(NB):                       # bottom=odd chunk of each block
            blk_k = k_s[b, h, g * BUCKET:(g + 1) * BUCKET]      # [256, D]
            ksT[0:64, g * 128:(g + 1) * 128] = blk_k[0:128].T
            ksT[64:128, g * 128:(g + 1) * 128] = blk_k[128:256].T

        kpiT = np.zeros((128, 512), F32)          # packed Kpi^T
        for p in range(4):
            je, jo = 2 * p, 2 * p + 1
            kpiT[0:64, p * 128:(p + 1) * 128] = Kpi[je * 128:(je + 1) * 128].T
            if jo < NCH:
                kpiT[64:128, p * 128:(p + 1) * 128] = Kpi[jo * 128:(jo + 1) * 128].T

        ws = np.ascontiguousarray(
            w_s[b, h].reshape(32, 128, D + 1).transpose(1, 0, 2)
        ).astype(BF16)                            # [128, 32, 65]

        wpim = np.zeros((128, NB * NCH, D + 1), BF16)  # 16 masked variants x 7 chunks
        for g in range(NB):
            Wm = Wpi.copy()
            Wm[bnd[g]:bnd[g + 1]] = 0.0
            Wm = Wm.reshape(NCH, 128, D + 1).transpose(1, 0, 2)   # [128, 7, 65]
            wpim[:, g * NCH:(g + 1) * NCH, :] = Wm.astype(BF16)

        per_bh.append(dict(qsT=qsT, ksT=ksT, kpiT=kpiT, ws=ws, wpim=wpim,
                           q_idx=q_idx[b, h]))
    return per_bh


def _build_nc():
    """Build the SPMD Bass program (one core's view: BH_PER_CORE heads)."""
    import concourse.mybir as mybir
    import concourse.tile as tile
    from concourse import bacc

    f32 = mybir.dt.float32
    bf16 = mybir.dt.bfloat16
    EXP = mybir.ActivationFunctionType.Exp

    nc = bacc.Bacc(None)
    qsT_d = nc.declare_dram_parameter("qsT", [BH_PER_CORE, 128, N], f32, isOutput=False)
    ksT_d = nc.declare_dram_parameter("ksT", [BH_PER_CORE, 128, NB * 128], f32, isOutput=False)
    kpiT_d = nc.declare_dram_parameter("kpiT", [BH_PER_CORE, 128, 512], f32, isOutput=False)
    ws_d = nc.declare_dram_parameter("ws", [BH_PER_CORE, 128, 32, D + 1], bf16, isOutput=False)
    wpim_d = nc.declare_dram_parameter("wpim", [BH_PER_CORE, 128, NB * NCH, D + 1], bf16, isOutput=False)
    att_d = nc.declare_dram_parameter("attT", [BH_PER_CORE, D + 1, N], f32, isOutput=True)

    with tile.TileContext(nc) as tc:
        with (
            tc.tile_pool(name="const", bufs=1) as cpool,
            tc.tile_pool(name="a", bufs=6) as apool,
            tc.tile_pool(name="o", bufs=3) as opool,
            tc.tile_pool(name="s_ps", bufs=3, space="PSUM") as spsum,
            tc.tile_pool(name="att_ps", bufs=2, space="PSUM") as attpsum,
        ):
            for bh in range(BH_PER_CORE):
                qsT = cpool.tile([128, N], f32, tag=f"qsT{bh}")
                ksT = cpool.tile([128, NB * 128], f32, tag=f"ksT{bh}")
                kpiT = cpool.tile([128, 512], f32, tag=f"kpiT{bh}")
                ws = cpool.tile([128, 32, D + 1], bf16, tag=f"ws{bh}")
                wpim = cpool.tile([128, NB * NCH, D + 1], bf16, tag=f"wpim{bh}")
                nc.sync.dma_start(qsT[:], qsT_d[bh])
                nc.sync.dma_start(ksT[:], ksT_d[bh])
                nc.sync.dma_start(kpiT[:], kpiT_d[bh])
                nc.sync.dma_start(ws[:], ws_d[bh])
                nc.sync.dma_start(wpim[:], wpim_d[bh])

                for c in range(8):  # query groups of 512 (blocks 2c, 2c+1)
                    qlo = qsT[0:64, c * 512:(c + 1) * 512]
                    qhi = qsT[64:128, c * 512:(c + 1) * 512]

                    # ---- residual S^T = Kpi @ q^T, row-packed chunk pairs ----
                    a2 = []
                    for p in range(4):
                        ps = spsum.tile([128, 1024], f32, tag="s")
                        nc.tensor.matmul(
                            ps[:, 0:512],
                            kpiT[0:64, p * 128:(p + 1) * 128], qlo,
                            start=True, stop=True,
                        )
                        if p < 3:
                            nc.tensor.matmul(
                                ps[:, 512:1024],
                                kpiT[64:128, p * 128:(p + 1) * 128], qhi,
                                start=True, stop=True,
                            )
                        a = apool.tile([128, 1024], bf16, tag="a")
                        if p < 3:
                            nc.scalar.activation(a[:], ps[:], EXP)
                        else:
                            nc.scalar.activation(a[:, 0:512], ps[:, 0:512], EXP)
                        a2.append(a)

                    # ---- att^T += Wpi_masked^T @ exp(S^T) ----
                    # NOTE: start=True clears has_written for the WHOLE bank, so
                    # exactly one start=True per PSUM bank; later first-writes to
                    # other regions overwrite via their cleared has_written bits.
                    att_ps = attpsum.tile([D + 1, 512], f32, tag="att")
                    for p in range(4):
                        for hh in range(2):
                            g = 2 * c + hh
                            je, jo = 2 * p, 2 * p + 1
                            nc.tensor.matmul(
                                att_ps[:, hh * 256:(hh + 1) * 256],
                                wpim[:, g * NCH + je, :],
                                a2[p][:, hh * 256 + 0:hh * 256 + 256],
                                start=(p == 0 and hh == 0), stop=False,
                            )
                            if jo < NCH:
                                nc.tensor.matmul(
                                    att_ps[:, hh * 256:(hh + 1) * 256],
                                    wpim[:, g * NCH + jo, :],
                                    a2[p][:, 512 + hh * 256:512 + hh * 256 + 256],
                                    start=False, stop=False,
                                )

                    # ---- block-sparse attention for blocks 2c, 2c+1 ----
                    psb = spsum.tile([128, 1024], f32, tag="s")
                    for hh in range(2):
                        g = 2 * c + hh
                        nc.tensor.matmul(
                            psb[:, hh * 256:hh * 256 + 256],
                            ksT[0:64, g * 128:(g + 1) * 128],
                            qsT[0:64, g * 256:(g + 1) * 256],
                            start=(hh == 0), stop=(hh == 1),
                        )
                        nc.tensor.matmul(
                            psb[:, 512 + hh * 256:512 + hh * 256 + 256],
                            ksT[64:128, g * 128:(g + 1) * 128],
                            qsT[64:128, g * 256:(g + 1) * 256],
                            start=(hh == 0), stop=(hh == 1),
                        )
                    a1 = apool.tile([128, 1024], bf16, tag="a")
                    nc.scalar.activation(a1[:], psb[:], EXP)
                    for hh in range(2):
                        g = 2 * c + hh
                        nc.tensor.matmul(
                            att_ps[:, hh * 256:(hh + 1) * 256],
                            ws[:, 2 * g, :],
                            a1[:, hh * 256:hh * 256 + 256],
                            start=False, stop=False,
                        )
                        nc.tensor.matmul(
                            att_ps[:, hh * 256:(hh + 1) * 256],
                            ws[:, 2 * g + 1, :],
                            a1[:, 512 + hh * 256:512 + hh * 256 + 256],
                            start=False, stop=(hh == 1),
                        )

                    att_sb = opool.tile([D + 1, 512], f32, tag="o")
                    nc.vector.tensor_copy(att_sb[:], att_ps[:])
                    nc.sync.dma_start(att_d[bh, :, c * 512:(c + 1) * 512], att_sb[:])
    nc.compile()
    return nc


def kernel(query, key, value, proj_dir, pm_init):
    global LAST_RESULT
    _ensure_ntff_hook()
    from concourse.bass_utils import run_bass_kernel_spmd

    query = np.asarray(query, F32)
    key = np.asarray(key, F32)
    value = np.asarray(value, F32)
    proj_dir = np.asarray(proj_dir, F32)
    pm_init = np.asarray(pm_init, F32)

    per_bh = _host_prep(query, key, value, proj_dir, pm_init)

    if "nc" not in _NC_CACHE:
        _NC_CACHE["nc"] = _build_nc()
    nc = _NC_CACHE["nc"]

    in_maps = []
    for core in range(NCORES):
        chunk = per_bh[core * BH_PER_CORE:(core + 1) * BH_PER_CORE]
        in_maps.append({
            "qsT": np.stack([d["qsT"] for d in chunk]),
            "ksT": np.stack([d["ksT"] for d in chunk]),
            "kpiT": np.stack([d["kpiT"] for d in chunk]),
            "ws": np.stack([d["ws"] for d in chunk]),
            "wpim": np.stack([d["wpim"] for d in chunk]),
        })

    trace = os.environ.get("CHA_TRACE", "0") == "1"
    res = run_bass_kernel_spmd(nc, in_maps, core_ids=list(range(NCORES)), trace=trace)
    LAST_RESULT = res

    est = np.empty((B, H, N, D), F32)
    for core in range(NCORES):
        att = res.results[core]["attT"]  # [BH_PER_CORE, 65, N]
        for j in range(BH_PER_CORE):
            bh = core * BH_PER_CORE + j
            b, h = bh // H, bh % H
            es = (att[j, :D, :] / att[j, D, :]).T      # [N, D] sorted order
            est[b, h, per_bh[bh]["q_idx"]] = es
    return est


# revision 5
# speedup vs baseline: 1.2985x; 1.2985x over previous
"""CosineHammingAttention on 8 TRN2 NeuronCores.

Host (cheap, control-flow heavy): LSH hashing, stable sort, gathers, power
method, sampling probabilities + jax categorical sampling, masked-weight
variant construction, final unsort/divide.

Device (per (b,h), 2 per core): block-sparse attention over sorted buckets +
importance-sampled residual attention, fused into one accumulated attention
tensor per query group.

Layout notes (per (b,h)):
  - all S = Q.K^T matmuls are computed transposed (S^T = [keys/samples, queries])
    so the exp(S^T) output feeds the A^T @ W matmul directly as the moving
    operand with contraction on partitions.
  - K=64 contractions are row-packed: two independent matmuls run concurrently
    in PE row-groups (0,0)/(64,0), writing different PSUM banks.
  - the (query-block, sample) mask is folded into 16 pre-masked copies of the
    sampled value matrix (exact zeros), so exp needs no per-op bias and m2 is
    a plain matmul.
"""

import os
import sys
import types

import numpy as np
import ml_dtypes

B, N, H, D = 2, 4096, 8, 64
NUM_PROJS, BUCKET, SAMPLE, PM_ITERS = 7, 256, 800, 32
NB = N // BUCKET          # 16 blocks
SPAD = 896                # samples padded to 7*128
NCH = SPAD // 128         # 7 sample chunks
NCORES = 8
BH_PER_CORE = (B * H) // NCORES  # 2

F32 = np.float32
BF16 = ml_dtypes.bfloat16

_NC_CACHE = {}
LAST_RESULT = None  # BassKernelResults of the most recent kernel() call


def _ensure_ntff_hook():
    """Register the axon NTFF profile hook if the image's antenv lacks it."""
    try:
        import antenv.axon_hooks  # noqa: F401
        return
    except ImportError:
        pass
    mod = types.ModuleType("antenv.axon_hooks")
    mod._hook = None
    def _set(h):
        mod._hook = h
    def _get():
        return mod._hook
    mod.set_axon_ntff_profile_hook = _set
    mod.get_axon_ntff_profile_hook = _get
    sys.modules["antenv.axon_hooks"] = mod
    try:
        from trn_agent_boot.trn_boot import _ntff_profile_via_ctypes
        hook = _ntff_profile_via_ctypes("/opt/axon/libaxon_pjrt.so")
        if hook is not None:
            _set(hook)
    except Exception:
        pass


def _hamming_perm(n):
    a = np.array([0, 1], dtype=np.int64)
    for s in range(2, n + 1):
        a = np.concatenate([a, a[::-1] + 2 ** (s - 1)])
    return a


def _host_prep(query, key, value, proj_dir, pm_init):
    """Hash/sort/sample on host. Returns per-(b,h) device inputs + unsort info."""
    import jax
    import jax.numpy as jnp

    cpu = jax.devices("cpu")[0]
    with jax.default_device(cpu):
        q = np.transpose(query, (0, 2, 1, 3)).astype(F32)   # [B,H,N,D]
        k = np.transpose(key, (0, 2, 1, 3)).astype(F32)
        v = np.transpose(value, (0, 2, 1, 3)).astype(F32)

        perm = _hamming_perm(NUM_PROJS)
        pow2 = (2 ** np.arange(NUM_PROJS)).astype(np.int64)

        def lsh(x):
            proj = np.asarray(
                jnp.einsum("bhnd,bhdr->bhnr", jnp.asarray(x), jnp.asarray(proj_dir))
            )
            bits = (proj > 0).astype(np.int64)
            return perm[(bits * pow2).sum(-1)]

        k_idx = np.argsort(lsh(k), axis=2, kind="stable")
        q_idx = np.argsort(lsh(q), axis=2, kind="stable")

        v_aug = np.concatenate([v, np.ones((B, H, N, 1), F32)], axis=-1)
        gat = lambda m, idx: np.take_along_axis(m, idx[..., None], axis=2)
        q_s = gat(q, q_idx)
        k_s = gat(k, k_idx)
        w_s = gat(v_aug, k_idx)

        # power method for the value-Gram spectral norm (matches reference jnp)
        w_j = jnp.asarray(w_s)
        Gram = jnp.einsum("bhnt,bhnd->bhtd", w_j, w_j)
        x = jnp.asarray(pm_init)
        x = x / jnp.linalg.norm(x, axis=2, keepdims=True)
        for _ in range(PM_ITERS):
            y = jnp.einsum("bhnm,bhm->bhn", Gram, x)
            x = y / jnp.linalg.norm(y, axis=2, keepdims=True)
        sigma = jnp.linalg.norm(y, axis=2)

        P = jnp.linalg.norm(w_j, axis=3) / sigma[:, :, None] + 1.0 / N
        P = jnp.maximum(P, 0.0)
        P = P / jnp.sum(P, axis=2, keepdims=True)

        skey = jax.random.key(1234)
        idx = jax.random.categorical(
            skey, jnp.log(P.reshape(B * H, N)), axis=-1, shape=(SAMPLE, B * H)
        ).T
        idx = np.asarray(idx)          # [BH, SAMPLE] sorted-key positions
        P = np.asarray(P)

    per_bh = []
    for bh in range(B * H):
        b, h = bh // H, bh % H
        i = idx[bh]
        blk = i // BUCKET
        order = np.argsort(blk, kind="stable")
        i_s = i[order]
        counts = np.bincount(blk[order], minlength=NB)
        bnd = np.zeros(NB + 1, np.int64)
        bnd[1:] = np.cumsum(counts)

        Kpi = np.zeros((SPAD, D), F32)
        Kpi[:SAMPLE] = k_s[b, h, i_s]
        sig = 1.0 / (P[b, h, i_s] * SAMPLE)
        Wpi = np.zeros((SPAD, D + 1), F32)
        Wpi[:SAMPLE] = sig[:, None] * w_s[b, h, i_s]

        # --- device layouts ---
        qsT = np.empty((128, N), F32)             # duplicated q_s^T (row packing)
        qsT[0:64] = q_s[b, h].T
        qsT[64:128] = qsT[0:64]

        ksT = np.zeros((128, NB * 128), F32)      # packed k_s^T: top=even chunk,
        for g in range(NB):                       # bottom=odd chunk of each block
            blk_k = k_s[b, h, g * BUCKET:(g + 1) * BUCKET]      # [256, D]
            ksT[0:64, g * 128:(g + 1) * 128] = blk_k[0:128].T
            ksT[64:128, g * 128:(g + 1) * 128] = blk_k[128:256].T

        kpiT = np.zeros((128, 512), F32)          # packed Kpi^T
        for p in range(4):
            je, jo = 2 * p, 2 * p + 1
            kpiT[0:64, p * 128:(p + 1) * 128] = Kpi[je * 128:(je + 1) * 128].T
            if jo < NCH:
                kpiT[64:128, p * 128:(p + 1) * 128] = Kpi[jo * 128:(jo + 1) * 128].T

        ws = np.ascontiguousarray(
            w_s[b, h].reshape(32, 128, D + 1).transpose(1, 0, 2)
        ).astype(BF16)                            # [128, 32, 65]

        wpim = np.zeros((128, NB * NCH, D + 1), BF16)  # 16 masked variants x 7 chunks
        for g in range(NB):
            Wm = Wpi.copy()
            Wm[bnd[g]:bnd[g + 1]] = 0.0
            Wm = Wm.reshape(NCH, 128, D + 1).transpose(1, 0, 2)   # [128, 7, 65]
            wpim[:, g * NCH:(g + 1) * NCH, :] = Wm.astype(BF16)

        per_bh.append(dict(qsT=qsT, ksT=ksT, kpiT=kpiT, ws=ws, wpim=wpim,
                           q_idx=q_idx[b, h]))
    return per_bh


def _build_nc():
    """Build the SPMD Bass program (one core's view: BH_PER_CORE heads)."""
    import concourse.mybir as mybir
    import concourse.tile as tile
    from concourse import bacc

    f32 = mybir.dt.float32
    f32r = mybir.dt.float32r   # fp32 storage, reduced-precision PE mode: 1 cyc/col
    bf16 = mybir.dt.bfloat16
    EXP = mybir.ActivationFunctionType.Exp

    nc = bacc.Bacc(None)
    qsT_d = nc.declare_dram_parameter("qsT", [BH_PER_CORE, 128, N], f32r, isOutput=False)
    ksT_d = nc.declare_dram_parameter("ksT", [BH_PER_CORE, 128, NB * 128], f32r, isOutput=False)
    kpiT_d = nc.declare_dram_parameter("kpiT", [BH_PER_CORE, 128, 512], f32r, isOutput=False)
    ws_d = nc.declare_dram_parameter("ws", [BH_PER_CORE, 128, 32, D + 1], bf16, isOutput=False)
    wpim_d = nc.declare_dram_parameter("wpim", [BH_PER_CORE, 128, NB * NCH, D + 1], bf16, isOutput=False)
    att_d = nc.declare_dram_parameter("attT", [BH_PER_CORE, D + 1, N], f32, isOutput=True)

    with tile.TileContext(nc) as tc:
        with (
            tc.tile_pool(name="const", bufs=1) as cpool,
            tc.tile_pool(name="a", bufs=6) as apool,
            tc.tile_pool(name="o", bufs=3) as opool,
            tc.tile_pool(name="s_ps", bufs=3, space="PSUM") as spsum,
            tc.tile_pool(name="att_ps", bufs=2, space="PSUM") as attpsum,
        ):
            for bh in range(BH_PER_CORE):
                qsT = cpool.tile([128, N], f32r, tag=f"qsT{bh}")
                ksT = cpool.tile([128, NB * 128], f32r, tag=f"ksT{bh}")
                kpiT = cpool.tile([128, 512], f32r, tag=f"kpiT{bh}")
                ws = cpool.tile([128, 32, D + 1], bf16, tag=f"ws{bh}")
                wpim = cpool.tile([128, NB * NCH, D + 1], bf16, tag=f"wpim{bh}")
                nc.sync.dma_start(qsT[:], qsT_d[bh])
                nc.sync.dma_start(ksT[:], ksT_d[bh])
                nc.sync.dma_start(kpiT[:], kpiT_d[bh])
                nc.sync.dma_start(ws[:], ws_d[bh])
                nc.sync.dma_start(wpim[:], wpim_d[bh])

                for c in range(8):  # query groups of 512 (blocks 2c, 2c+1)
                    qlo = qsT[0:64, c * 512:(c + 1) * 512]
                    qhi = qsT[64:128, c * 512:(c + 1) * 512]

                    # ---- residual S^T = Kpi @ q^T, row-packed chunk pairs ----
                    a2 = []
                    for p in range(4):
                        ps = spsum.tile([128, 1024], f32, tag="s")
                        nc.tensor.matmul(
                            ps[:, 0:512],
                            kpiT[0:64, p * 128:(p + 1) * 128], qlo,
                            start=True, stop=True,
                        )
                        if p < 3:
                            nc.tensor.matmul(
                                ps[:, 512:1024],
                                kpiT[64:128, p * 128:(p + 1) * 128], qhi,
                                start=True, stop=True,
                            )
                        a = apool.tile([128, 1024], bf16, tag="a")
                        if p < 3:
                            nc.scalar.activation(a[:], ps[:], EXP)
                        else:
                            nc.scalar.activation(a[:, 0:512], ps[:, 0:512], EXP)
                        a2.append(a)

                    # ---- att^T += Wpi_masked^T @ exp(S^T) ----
                    # NOTE: start=True clears has_written for the WHOLE bank, so
                    # exactly one start=True per PSUM bank; later first-writes to
                    # other regions overwrite via their cleared has_written bits.
                    att_ps = attpsum.tile([D + 1, 512], f32, tag="att")
                    for p in range(4):
                        for hh in range(2):
                            g = 2 * c + hh
                            je, jo = 2 * p, 2 * p + 1
                            nc.tensor.matmul(
                                att_ps[:, hh * 256:(hh + 1) * 256],
                                wpim[:, g * NCH + je, :],
                                a2[p][:, hh * 256 + 0:hh * 256 + 256],
                                start=(p == 0 and hh == 0), stop=False,
                            )
                            if jo < NCH:
                                nc.tensor.matmul(
                                    att_ps[:, hh * 256:(hh + 1) * 256],
                                    wpim[:, g * NCH + jo, :],
                                    a2[p][:, 512 + hh * 256:512 + hh * 256 + 256],
                                    start=False, stop=False,
                                )

                    # ---- block-sparse attention for blocks 2c, 2c+1 ----
                    psb = spsum.tile([128, 1024], f32, tag="s")
                    for hh in range(2):
                        g = 2 * c + hh
                        nc.tensor.matmul(
                            psb[:, hh * 256:hh * 256 + 256],
                            ksT[0:64, g * 128:(g + 1) * 128],
                            qsT[0:64, g * 256:(g + 1) * 256],
                            start=(hh == 0), stop=(hh == 1),
                        )
                        nc.tensor.matmul(
                            psb[:, 512 + hh * 256:512 + hh * 256 + 256],
                            ksT[64:128, g * 128:(g + 1) * 128],
                            qsT[64:128, g * 256:(g + 1) * 256],
                            start=(hh == 0), stop=(hh == 1),
                        )
                    a1 = apool.tile([128, 1024], bf16, tag="a")
                    nc.scalar.activation(a1[:], psb[:], EXP)
                    for hh in range(2):
                        g = 2 * c + hh
                        nc.tensor.matmul(
                            att_ps[:, hh * 256:(hh + 1) * 256],
                            ws[:, 2 * g, :],
                            a1[:, hh * 256:hh * 256 + 256],
                            start=False, stop=False,
                        )
                        nc.tensor.matmul(
                            att_ps[:, hh * 256:(hh + 1) * 256],
                            ws[:, 2 * g + 1, :],
                            a1[:, 512 + hh * 256:512 + hh * 256 + 256],
                            start=False, stop=(hh == 1),
                        )

                    att_sb = opool.tile([D + 1, 512], f32, tag="o")
                    nc.vector.tensor_copy(att_sb[:], att_ps[:])
                    nc.sync.dma_start(att_d[bh, :, c * 512:(c + 1) * 512], att_sb[:])
    nc.compile()
    return nc


def kernel(query, key, value, proj_dir, pm_init):
    global LAST_RESULT
    _ensure_ntff_hook()
    from concourse.bass_utils import run_bass_kernel_spmd

    query = np.asarray(query, F32)
    key = np.asarray(key, F32)
    value = np.asarray(value, F32)
    proj_dir = np.asarray(proj_dir, F32)
    pm_init = np.asarray(pm_init, F32)

    per_bh = _host_prep(query, key, value, proj_dir, pm_init)

    if "nc" not in _NC_CACHE:
        _NC_CACHE["nc"] = _build_nc()
    nc = _NC_CACHE["nc"]

    in_maps = []
    for core in range(NCORES):
        chunk = per_bh[core * BH_PER_CORE:(core + 1) * BH_PER_CORE]
        in_maps.append({
            "qsT": np.stack([d["qsT"] for d in chunk]),
            "ksT": np.stack([d["ksT"] for d in chunk]),
            "kpiT": np.stack([d["kpiT"] for d in chunk]),
            "ws": np.stack([d["ws"] for d in chunk]),
            "wpim": np.stack([d["wpim"] for d in chunk]),
        })

    trace = os.environ.get("CHA_TRACE", "0") == "1"
    res = run_bass_kernel_spmd(nc, in_maps, core_ids=list(range(NCORES)), trace=trace)
    LAST_RESULT = res

    est = np.empty((B, H, N, D), F32)
    for core in range(NCORES):
        att = res.results[core]["attT"]  # [BH_PER_CORE, 65, N]
        for j in range(BH_PER_CORE):
            bh = core * BH_PER_CORE + j
            b, h = bh // H, bh % H
            es = (att[j, :D, :] / att[j, D, :]).T      # [N, D] sorted order
            est[b, h, per_bh[bh]["q_idx"]] = es
    return est


# revision 6
# speedup vs baseline: 1.3851x; 1.0668x over previous
"""CosineHammingAttention on 8 TRN2 NeuronCores.

Host (cheap, control-flow heavy): LSH hashing, stable sort, gathers, power
method, sampling probabilities + jax categorical sampling, masked-weight
variant construction, final unsort/divide.

Device (per (b,h), 2 per core): block-sparse attention over sorted buckets +
importance-sampled residual attention, fused into one accumulated attention
tensor per query group.

Layout notes (per (b,h)):
  - all S = Q.K^T matmuls are computed transposed (S^T = [keys/samples, queries])
    so the exp(S^T) output feeds the A^T @ W matmul directly as the moving
    operand with contraction on partitions.
  - K=64 contractions are row-packed: two independent matmuls run concurrently
    in PE row-groups (0,0)/(64,0), writing different PSUM banks.
  - the (query-block, sample) mask is folded into 16 pre-masked copies of the
    sampled value matrix (exact zeros), so exp needs no per-op bias and m2 is
    a plain matmul.
"""

import os
import sys
import types

import numpy as np
import ml_dtypes

B, N, H, D = 2, 4096, 8, 64
NUM_PROJS, BUCKET, SAMPLE, PM_ITERS = 7, 256, 800, 32
NB = N // BUCKET          # 16 blocks
SPAD = 896                # samples padded to 7*128
NCH = SPAD // 128         # 7 sample chunks
NCORES = 8
BH_PER_CORE = (B * H) // NCORES  # 2

F32 = np.float32
BF16 = ml_dtypes.bfloat16

_NC_CACHE = {}
LAST_RESULT = None  # BassKernelResults of the most recent kernel() call


def _ensure_ntff_hook():
    """Register the axon NTFF profile hook if the image's antenv lacks it."""
    try:
        import antenv.axon_hooks  # noqa: F401
        return
    except ImportError:
        pass
    mod = types.ModuleType("antenv.axon_hooks")
    mod._hook = None
    def _set(h):
        mod._hook = h
    def _get():
        return mod._hook
    mod.set_axon_ntff_profile_hook = _set
    mod.get_axon_ntff_profile_hook = _get
    sys.modules["antenv.axon_hooks"] = mod
    try:
        from trn_agent_boot.trn_boot import _ntff_profile_via_ctypes
        hook = _ntff_profile_via_ctypes("/opt/axon/libaxon_pjrt.so")
        if hook is not None:
            _set(hook)
    except Exception:
        pass


def _hamming_perm(n):
    a = np.array([0, 1], dtype=np.int64)
    for s in range(2, n + 1):
        a = np.concatenate([a, a[::-1] + 2 ** (s - 1)])
    return a


def _host_prep(query, key, value, proj_dir, pm_init):
    """Hash/sort/sample on host. Returns per-(b,h) device inputs + unsort info."""
    import jax
    import jax.numpy as jnp

    cpu = jax.devices("cpu")[0]
    with jax.default_device(cpu):
        q = np.transpose(query, (0, 2, 1, 3)).astype(F32)   # [B,H,N,D]
        k = np.transpose(key, (0, 2, 1, 3)).astype(F32)
        v = np.transpose(value, (0, 2, 1, 3)).astype(F32)

        perm = _hamming_perm(NUM_PROJS)
        pow2 = (2 ** np.arange(NUM_PROJS)).astype(np.int64)

        def lsh(x):
            proj = np.asarray(
                jnp.einsum("bhnd,bhdr->bhnr", jnp.asarray(x), jnp.asarray(proj_dir))
            )
            bits = (proj > 0).astype(np.int64)
            return perm[(bits * pow2).sum(-1)]

        k_idx = np.argsort(lsh(k), axis=2, kind="stable")
        q_idx = np.argsort(lsh(q), axis=2, kind="stable")

        v_aug = np.concatenate([v, np.ones((B, H, N, 1), F32)], axis=-1)
        gat = lambda m, idx: np.take_along_axis(m, idx[..., None], axis=2)
        q_s = gat(q, q_idx)
        k_s = gat(k, k_idx)
        w_s = gat(v_aug, k_idx)

        # power method for the value-Gram spectral norm (matches reference jnp)
        w_j = jnp.asarray(w_s)
        Gram = jnp.einsum("bhnt,bhnd->bhtd", w_j, w_j)
        x = jnp.asarray(pm_init)
        x = x / jnp.linalg.norm(x, axis=2, keepdims=True)
        for _ in range(PM_ITERS):
            y = jnp.einsum("bhnm,bhm->bhn", Gram, x)
            x = y / jnp.linalg.norm(y, axis=2, keepdims=True)
        sigma = jnp.linalg.norm(y, axis=2)

        P = jnp.linalg.norm(w_j, axis=3) / sigma[:, :, None] + 1.0 / N
        P = jnp.maximum(P, 0.0)
        P = P / jnp.sum(P, axis=2, keepdims=True)

        skey = jax.random.key(1234)
        idx = jax.random.categorical(
            skey, jnp.log(P.reshape(B * H, N)), axis=-1, shape=(SAMPLE, B * H)
        ).T
        idx = np.asarray(idx)          # [BH, SAMPLE] sorted-key positions
        P = np.asarray(P)

    per_bh = []
    for bh in range(B * H):
        b, h = bh // H, bh % H
        i = idx[bh]
        blk = i // BUCKET
        order = np.argsort(blk, kind="stable")
        i_s = i[order]
        counts = np.bincount(blk[order], minlength=NB)
        bnd = np.zeros(NB + 1, np.int64)
        bnd[1:] = np.cumsum(counts)

        Kpi = np.zeros((SPAD, D), F32)
        Kpi[:SAMPLE] = k_s[b, h, i_s]
        sig = 1.0 / (P[b, h, i_s] * SAMPLE)
        Wpi = np.zeros((SPAD, D + 1), F32)
        Wpi[:SAMPLE] = sig[:, None] * w_s[b, h, i_s]

        # --- device layouts ---
        qsT = np.empty((128, N), F32)             # duplicated q_s^T (row packing)
        qsT[0:64] = q_s[b, h].T
        qsT[64:128] = qsT[0:64]

        ksT = np.zeros((128, NB * 128), F32)      # packed k_s^T: top=even chunk,
        for g in range(NB):                       # bottom=odd chunk of each block
            blk_k = k_s[b, h, g * BUCKET:(g + 1) * BUCKET]      # [256, D]
            ksT[0:64, g * 128:(g + 1) * 128] = blk_k[0:128].T
            ksT[64:128, g * 128:(g + 1) * 128] = blk_k[128:256].T

        kpiT = np.zeros((128, 512), F32)          # packed Kpi^T
        for p in range(4):
            je, jo = 2 * p, 2 * p + 1
            kpiT[0:64, p * 128:(p + 1) * 128] = Kpi[je * 128:(je + 1) * 128].T
            if jo < NCH:
                kpiT[64:128, p * 128:(p + 1) * 128] = Kpi[jo * 128:(jo + 1) * 128].T

        # Fold the residual (query-block, sample) mask into the block weights:
        # each sampled key is a duplicate of a block key, and the mask removes
        # exactly the same-block sample terms, so subtracting the per-key sum
        # of sampled weights (wc) from the block-attention weights cancels them
        # (the exp factors are identical dot products on both paths).
        wc = np.zeros((N, D + 1), F32)
        np.add.at(wc, i_s, Wpi[:SAMPLE])
        wsc = np.ascontiguousarray(
            (w_s[b, h] - wc).reshape(32, 128, D + 1).transpose(1, 0, 2)
        ).astype(BF16)                            # [128, 32, 65]

        wpi = np.ascontiguousarray(
            Wpi.reshape(NCH, 128, D + 1).transpose(1, 0, 2)
        ).astype(BF16)                            # [128, 7, 65] unmasked

        per_bh.append(dict(qsT=qsT, ksT=ksT, kpiT=kpiT, ws=wsc, wpim=wpi,
                           q_idx=q_idx[b, h]))
    return per_bh


def _build_nc():
    """Build the SPMD Bass program (one core's view: BH_PER_CORE heads)."""
    import concourse.mybir as mybir
    import concourse.tile as tile
    from concourse import bacc

    f32 = mybir.dt.float32
    f32r = mybir.dt.float32r   # fp32 storage, reduced-precision PE mode: 1 cyc/col
    bf16 = mybir.dt.bfloat16
    EXP = mybir.ActivationFunctionType.Exp

    nc = bacc.Bacc(None)
    qsT_d = nc.declare_dram_parameter("qsT", [BH_PER_CORE, 128, N], f32r, isOutput=False)
    ksT_d = nc.declare_dram_parameter("ksT", [BH_PER_CORE, 128, NB * 128], f32r, isOutput=False)
    kpiT_d = nc.declare_dram_parameter("kpiT", [BH_PER_CORE, 128, 512], f32r, isOutput=False)
    ws_d = nc.declare_dram_parameter("ws", [BH_PER_CORE, 128, 32, D + 1], bf16, isOutput=False)
    wpim_d = nc.declare_dram_parameter("wpim", [BH_PER_CORE, 128, NCH, D + 1], bf16, isOutput=False)
    att_d = nc.declare_dram_parameter("attT", [BH_PER_CORE, D + 1, N], f32, isOutput=True)

    with tile.TileContext(nc) as tc:
        with (
            tc.tile_pool(name="const", bufs=1) as cpool,
            tc.tile_pool(name="a", bufs=6) as apool,
            tc.tile_pool(name="o", bufs=3) as opool,
            tc.tile_pool(name="s_ps", bufs=3, space="PSUM") as spsum,
            tc.tile_pool(name="att_ps", bufs=2, space="PSUM") as attpsum,
        ):
            for bh in range(BH_PER_CORE):
                qsT = cpool.tile([128, N], f32r, tag=f"qsT{bh}")
                ksT = cpool.tile([128, NB * 128], f32r, tag=f"ksT{bh}")
                kpiT = cpool.tile([128, 512], f32r, tag=f"kpiT{bh}")
                ws = cpool.tile([128, 32, D + 1], bf16, tag=f"ws{bh}")
                wpim = cpool.tile([128, NCH, D + 1], bf16, tag=f"wpim{bh}")
                # split + order the loads so the first query groups can start
                # while the rest of the inputs stream in
                nc.sync.dma_start(kpiT[:], kpiT_d[bh])
                nc.sync.dma_start(qsT[:, 0:1024], qsT_d[bh, :, 0:1024])
                nc.sync.dma_start(wpim[:], wpim_d[bh])
                nc.sync.dma_start(ksT[:, 0:512], ksT_d[bh, :, 0:512])
                nc.sync.dma_start(ws[:, 0:8, :], ws_d[bh, :, 0:8, :])
                nc.sync.dma_start(qsT[:, 1024:2048], qsT_d[bh, :, 1024:2048])
                nc.sync.dma_start(ksT[:, 512:1024], ksT_d[bh, :, 512:1024])
                nc.sync.dma_start(ws[:, 8:16, :], ws_d[bh, :, 8:16, :])
                nc.sync.dma_start(qsT[:, 2048:4096], qsT_d[bh, :, 2048:4096])
                nc.sync.dma_start(ksT[:, 1024:2048], ksT_d[bh, :, 1024:2048])
                nc.sync.dma_start(ws[:, 16:32, :], ws_d[bh, :, 16:32, :])

                for c in range(8):  # query groups of 512 (blocks 2c, 2c+1)
                    qlo = qsT[0:64, c * 512:(c + 1) * 512]
                    qhi = qsT[64:128, c * 512:(c + 1) * 512]

                    # ---- residual S^T = Kpi @ q^T, row-packed chunk pairs ----
                    a2 = []
                    for p in range(4):
                        ps = spsum.tile([128, 1024], f32, tag="s")
                        nc.tensor.matmul(
                            ps[:, 0:512],
                            kpiT[0:64, p * 128:(p + 1) * 128], qlo,
                            start=True, stop=True,
                        )
                        if p < 3:
                            nc.tensor.matmul(
                                ps[:, 512:1024],
                                kpiT[64:128, p * 128:(p + 1) * 128], qhi,
                                start=True, stop=True,
                            )
                        a = apool.tile([128, 1024], bf16, tag="a")
                        if p < 3:
                            nc.scalar.activation(a[:], ps[:], EXP)
                        else:
                            nc.scalar.activation(a[:, 0:512], ps[:, 0:512], EXP)
                        a2.append(a)

                    # ---- att^T += Wpi_masked^T @ exp(S^T) ----
                    # NOTE: start=True clears has_written for the WHOLE bank, so
                    # exactly one start=True per PSUM bank; later first-writes to
                    # other regions overwrite via their cleared has_written bits.
                    att_ps = attpsum.tile([D + 1, 512], f32, tag="att")
                    for j in range(NCH):
                        p, odd = j // 2, j % 2
                        nc.tensor.matmul(
                            att_ps[:],
                            wpim[:, j, :],
                            a2[p][:, odd * 512:odd * 512 + 512],
                            start=(j == 0), stop=False,
                        )

                    # ---- block-sparse attention for blocks 2c, 2c+1 ----
                    psb = spsum.tile([128, 1024], f32, tag="s")
                    for hh in range(2):
                        g = 2 * c + hh
                        nc.tensor.matmul(
                            psb[:, hh * 256:hh * 256 + 256],
                            ksT[0:64, g * 128:(g + 1) * 128],
                            qsT[0:64, g * 256:(g + 1) * 256],
                            start=(hh == 0), stop=(hh == 1),
                        )
                        nc.tensor.matmul(
                            psb[:, 512 + hh * 256:512 + hh * 256 + 256],
                            ksT[64:128, g * 128:(g + 1) * 128],
                            qsT[64:128, g * 256:(g + 1) * 256],
                            start=(hh == 0), stop=(hh == 1),
                        )
                    a1 = apool.tile([128, 1024], bf16, tag="a")
                    nc.scalar.activation(a1[:], psb[:], EXP)
                    for hh in range(2):
                        g = 2 * c + hh
                        nc.tensor.matmul(
                            att_ps[:, hh * 256:(hh + 1) * 256],
                            ws[:, 2 * g, :],
                            a1[:, hh * 256:hh * 256 + 256],
                            start=False, stop=False,
                        )
                        nc.tensor.matmul(
                            att_ps[:, hh * 256:(hh + 1) * 256],
                            ws[:, 2 * g + 1, :],
                            a1[:, 512 + hh * 256:512 + hh * 256 + 256],
                            start=False, stop=(hh == 1),
                        )

                    att_sb = opool.tile([D + 1, 512], f32, tag="o")
                    nc.vector.tensor_copy(att_sb[:], att_ps[:])
                    nc.sync.dma_start(att_d[bh, :, c * 512:(c + 1) * 512], att_sb[:])
    nc.compile()
    return nc


def kernel(query, key, value, proj_dir, pm_init):
    global LAST_RESULT
    _ensure_ntff_hook()
    from concourse.bass_utils import run_bass_kernel_spmd

    query = np.asarray(query, F32)
    key = np.asarray(key, F32)
    value = np.asarray(value, F32)
    proj_dir = np.asarray(proj_dir, F32)
    pm_init = np.asarray(pm_init, F32)

    per_bh = _host_prep(query, key, value, proj_dir, pm_init)

    if "nc" not in _NC_CACHE:
        _NC_CACHE["nc"] = _build_nc()
    nc = _NC_CACHE["nc"]

    in_maps = []
    for core in range(NCORES):
        chunk = per_bh[core * BH_PER_CORE:(core + 1) * BH_PER_CORE]
        in_maps.append({
            "qsT": np.stack([d["qsT"] for d in chunk]),
            "ksT": np.stack([d["ksT"] for d in chunk]),
            "kpiT": np.stack([d["kpiT"] for d in chunk]),
            "ws": np.stack([d["ws"] for d in chunk]),
            "wpim": np.stack([d["wpim"] for d in chunk]),
        })

    trace = os.environ.get("CHA_TRACE", "0") == "1"
    res = run_bass_kernel_spmd(nc, in_maps, core_ids=list(range(NCORES)), trace=trace)
    LAST_RESULT = res

    est = np.empty((B, H, N, D), F32)
    for core in range(NCORES):
        att = res.results[core]["attT"]  # [BH_PER_CORE, 65, N]
        for j in range(BH_PER_CORE):
            bh = core * BH_PER_CORE + j
            b, h = bh // H, bh % H
            es = (att[j, :D, :] / att[j, D, :]).T      # [N, D] sorted order
            est[b, h, per_bh[bh]["q_idx"]] = es
    return est


# revision 8
# speedup vs baseline: 1.5505x; 1.1193x over previous
"""CosineHammingAttention on 8 TRN2 NeuronCores.

Host (cheap, control-flow heavy): LSH hashing, stable sort, gathers, power
method, sampling probabilities + jax categorical sampling, masked-weight
variant construction, final unsort/divide.

Device (per (b,h), 2 per core): block-sparse attention over sorted buckets +
importance-sampled residual attention, fused into one accumulated attention
tensor per query group.

Layout notes (per (b,h)):
  - all S = Q.K^T matmuls are computed transposed (S^T = [keys/samples, queries])
    so the exp(S^T) output feeds the A^T @ W matmul directly as the moving
    operand with contraction on partitions.
  - K=64 contractions are row-packed: two independent matmuls run concurrently
    in PE row-groups (0,0)/(64,0), writing different PSUM banks.
  - the (query-block, sample) mask is folded into 16 pre-masked copies of the
    sampled value matrix (exact zeros), so exp needs no per-op bias and m2 is
    a plain matmul.
"""

import os
import sys
import types

import numpy as np
import ml_dtypes

B, N, H, D = 2, 4096, 8, 64
NUM_PROJS, BUCKET, SAMPLE, PM_ITERS = 7, 256, 800, 32
NB = N // BUCKET          # 16 blocks
SPAD = 896                # samples padded to 7*128
NCH = SPAD // 128         # 7 sample chunks
NCORES = 8
BH_PER_CORE = (B * H) // NCORES  # 2

F32 = np.float32
BF16 = ml_dtypes.bfloat16

_NC_CACHE = {}
LAST_RESULT = None  # BassKernelResults of the most recent kernel() call


def _ensure_ntff_hook():
    """Register the axon NTFF profile hook if the image's antenv lacks it."""
    try:
        import antenv.axon_hooks  # noqa: F401
        return
    except ImportError:
        pass
    mod = types.ModuleType("antenv.axon_hooks")
    mod._hook = None
    def _set(h):
        mod._hook = h
    def _get():
        return mod._hook
    mod.set_axon_ntff_profile_hook = _set
    mod.get_axon_ntff_profile_hook = _get
    sys.modules["antenv.axon_hooks"] = mod
    try:
        from trn_agent_boot.trn_boot import _ntff_profile_via_ctypes
        hook = _ntff_profile_via_ctypes("/opt/axon/libaxon_pjrt.so")
        if hook is not None:
            _set(hook)
    except Exception:
        pass


def _hamming_perm(n):
    a = np.array([0, 1], dtype=np.int64)
    for s in range(2, n + 1):
        a = np.concatenate([a, a[::-1] + 2 ** (s - 1)])
    return a


def _host_prep(query, key, value, proj_dir, pm_init):
    """Hash/sort/sample on host. Returns per-(b,h) device inputs + unsort info."""
    import jax
    import jax.numpy as jnp

    cpu = jax.devices("cpu")[0]
    with jax.default_device(cpu):
        q = np.transpose(query, (0, 2, 1, 3)).astype(F32)   # [B,H,N,D]
        k = np.transpose(key, (0, 2, 1, 3)).astype(F32)
        v = np.transpose(value, (0, 2, 1, 3)).astype(F32)

        perm = _hamming_perm(NUM_PROJS)
        pow2 = (2 ** np.arange(NUM_PROJS)).astype(np.int64)

        def lsh(x):
            proj = np.asarray(
                jnp.einsum("bhnd,bhdr->bhnr", jnp.asarray(x), jnp.asarray(proj_dir))
            )
            bits = (proj > 0).astype(np.int64)
            return perm[(bits * pow2).sum(-1)]

        k_idx = np.argsort(lsh(k), axis=2, kind="stable")
        q_idx = np.argsort(lsh(q), axis=2, kind="stable")

        v_aug = np.concatenate([v, np.ones((B, H, N, 1), F32)], axis=-1)
        gat = lambda m, idx: np.take_along_axis(m, idx[..., None], axis=2)
        q_s = gat(q, q_idx)
        k_s = gat(k, k_idx)
        w_s = gat(v_aug, k_idx)

        # power method for the value-Gram spectral norm (matches reference jnp)
        w_j = jnp.asarray(w_s)
        Gram = jnp.einsum("bhnt,bhnd->bhtd", w_j, w_j)
        x = jnp.asarray(pm_init)
        x = x / jnp.linalg.norm(x, axis=2, keepdims=True)
        for _ in range(PM_ITERS):
            y = jnp.einsum("bhnm,bhm->bhn", Gram, x)
            x = y / jnp.linalg.norm(y, axis=2, keepdims=True)
        sigma = jnp.linalg.norm(y, axis=2)

        P = jnp.linalg.norm(w_j, axis=3) / sigma[:, :, None] + 1.0 / N
        P = jnp.maximum(P, 0.0)
        P = P / jnp.sum(P, axis=2, keepdims=True)

        skey = jax.random.key(1234)
        idx = jax.random.categorical(
            skey, jnp.log(P.reshape(B * H, N)), axis=-1, shape=(SAMPLE, B * H)
        ).T
        idx = np.asarray(idx)          # [BH, SAMPLE] sorted-key positions
        P = np.asarray(P)

    per_bh = []
    for bh in range(B * H):
        b, h = bh // H, bh % H
        i = idx[bh]
        blk = i // BUCKET
        order = np.argsort(blk, kind="stable")
        i_s = i[order]
        counts = np.bincount(blk[order], minlength=NB)
        bnd = np.zeros(NB + 1, np.int64)
        bnd[1:] = np.cumsum(counts)

        Kpi = np.zeros((SPAD, D), F32)
        Kpi[:SAMPLE] = k_s[b, h, i_s]
        sig = 1.0 / (P[b, h, i_s] * SAMPLE)
        Wpi = np.zeros((SPAD, D + 1), F32)
        Wpi[:SAMPLE] = sig[:, None] * w_s[b, h, i_s]

        # --- device layouts ---
        qsT = np.empty((128, N), F32)             # duplicated q_s^T (row packing)
        qsT[0:64] = q_s[b, h].T
        qsT[64:128] = qsT[0:64]

        ksT = np.zeros((128, NB * 128), F32)      # packed k_s^T: top=even chunk,
        for g in range(NB):                       # bottom=odd chunk of each block
            blk_k = k_s[b, h, g * BUCKET:(g + 1) * BUCKET]      # [256, D]
            ksT[0:64, g * 128:(g + 1) * 128] = blk_k[0:128].T
            ksT[64:128, g * 128:(g + 1) * 128] = blk_k[128:256].T

        kpiT = np.zeros((128, 512), F32)          # packed Kpi^T
        for p in range(4):
            je, jo = 2 * p, 2 * p + 1
            kpiT[0:64, p * 128:(p + 1) * 128] = Kpi[je * 128:(je + 1) * 128].T
            if jo < NCH:
                kpiT[64:128, p * 128:(p + 1) * 128] = Kpi[jo * 128:(jo + 1) * 128].T

        # Fold the residual (query-block, sample) mask into the block weights:
        # each sampled key is a duplicate of a block key, and the mask removes
        # exactly the same-block sample terms, so subtracting the per-key sum
        # of sampled weights (wc) from the block-attention weights cancels them
        # (the exp factors are identical dot products on both paths).
        wc = np.zeros((N, D + 1), F32)
        np.add.at(wc, i_s, Wpi[:SAMPLE])
        wsc = np.ascontiguousarray(
            (w_s[b, h] - wc).reshape(32, 128, D + 1).transpose(1, 0, 2)
        ).astype(BF16)                            # [128, 32, 65]

        wpi = np.ascontiguousarray(
            Wpi.reshape(NCH, 128, D + 1).transpose(1, 0, 2)
        ).astype(BF16)                            # [128, 7, 65] unmasked

        per_bh.append(dict(qsT=qsT, ksT=ksT, kpiT=kpiT, ws=wsc, wpim=wpi,
                           q_idx=q_idx[b, h]))
    return per_bh


def _build_nc():
    """Build the SPMD Bass program (one core's view: BH_PER_CORE heads)."""
    import concourse.mybir as mybir
    import concourse.tile as tile
    from concourse import bacc

    f32 = mybir.dt.float32
    f32r = mybir.dt.float32r   # fp32 storage, reduced-precision PE mode: 1 cyc/col
    bf16 = mybir.dt.bfloat16
    EXP = mybir.ActivationFunctionType.Exp

    nc = bacc.Bacc(None)
    qsT_d = nc.declare_dram_parameter("qsT", [BH_PER_CORE, 128, N], f32r, isOutput=False)
    ksT_d = nc.declare_dram_parameter("ksT", [BH_PER_CORE, 128, NB * 128], f32r, isOutput=False)
    kpiT_d = nc.declare_dram_parameter("kpiT", [BH_PER_CORE, 128, 512], f32r, isOutput=False)
    ws_d = nc.declare_dram_parameter("ws", [BH_PER_CORE, 128, 32, D + 1], bf16, isOutput=False)
    wpim_d = nc.declare_dram_parameter("wpim", [BH_PER_CORE, 128, NCH, D + 1], bf16, isOutput=False)
    att_d = nc.declare_dram_parameter("attT", [BH_PER_CORE, D + 1, N], f32, isOutput=True)

    with tile.TileContext(nc) as tc:
        with (
            tc.tile_pool(name="warm", bufs=1) as wpool,
            tc.tile_pool(name="const", bufs=1) as cpool,
            tc.tile_pool(name="a", bufs=6) as apool,
            tc.tile_pool(name="o", bufs=3) as opool,
            tc.tile_pool(name="s_ps", bufs=2, space="PSUM") as spsum,
            tc.tile_pool(name="att_ps", bufs=2, space="PSUM") as attpsum,
        ):
            wt = wpool.tile([1, 1], f32)
            nc.vector.memset(wt[:], 0.0)
            nc.scalar.activation(wt[:], wt[:], EXP)  # pull ACT table load off the critical path
            for bh in range(BH_PER_CORE):
                qsT = cpool.tile([128, N], f32r, tag=f"qsT{bh}")
                ksT = cpool.tile([128, NB * 128], f32r, tag=f"ksT{bh}")
                kpiT = cpool.tile([128, 512], f32r, tag=f"kpiT{bh}")
                ws = cpool.tile([128, 32, D + 1], bf16, tag=f"ws{bh}")
                wpim = cpool.tile([128, NCH, D + 1], bf16, tag=f"wpim{bh}")
                # split + order the loads so the first query groups can start
                # while the rest of the inputs stream in
                nc.sync.dma_start(kpiT[:], kpiT_d[bh])
                nc.sync.dma_start(qsT[:, 0:512], qsT_d[bh, :, 0:512])
                nc.sync.dma_start(qsT[:, 512:1024], qsT_d[bh, :, 512:1024])
                nc.sync.dma_start(wpim[:], wpim_d[bh])
                nc.sync.dma_start(ksT[:, 0:512], ksT_d[bh, :, 0:512])
                nc.sync.dma_start(ws[:, 0:8, :], ws_d[bh, :, 0:8, :])
                nc.sync.dma_start(qsT[:, 1024:2048], qsT_d[bh, :, 1024:2048])
                nc.sync.dma_start(ksT[:, 512:1024], ksT_d[bh, :, 512:1024])
                nc.sync.dma_start(ws[:, 8:16, :], ws_d[bh, :, 8:16, :])
                nc.sync.dma_start(qsT[:, 2048:4096], qsT_d[bh, :, 2048:4096])
                nc.sync.dma_start(ksT[:, 1024:2048], ksT_d[bh, :, 1024:2048])
                nc.sync.dma_start(ws[:, 16:32, :], ws_d[bh, :, 16:32, :])

                for c in range(8):  # query groups of 512 (blocks 2c, 2c+1)
                    qlo = qsT[0:64, c * 512:(c + 1) * 512]
                    qhi = qsT[64:128, c * 512:(c + 1) * 512]

                    # ---- S^T matmuls into three 3-bank tiles, one exp each ----
                    # T0: res chunks 0,1,2   T1: res chunks 3,4,5
                    # T2: res chunk 6 | blk kc0 (both blocks) | blk kc1
                    av = []
                    for t in range(2):
                        ps = spsum.tile([128, 1536], f32, tag="s")
                        for u in range(3):
                            j = 3 * t + u
                            if j % 2 == 0:
                                nc.tensor.matmul(
                                    ps[:, u * 512:(u + 1) * 512],
                                    kpiT[0:64, (j // 2) * 128:(j // 2 + 1) * 128],
                                    qlo, start=True, stop=True,
                                )
                            else:
                                nc.tensor.matmul(
                                    ps[:, u * 512:(u + 1) * 512],
                                    kpiT[64:128, (j // 2) * 128:(j // 2 + 1) * 128],
                                    qhi, start=True, stop=True,
                                )
                        a = apool.tile([128, 1536], bf16, tag="a")
                        nc.scalar.activation(a[:], ps[:], EXP)
                        av.append(a)

                    ps = spsum.tile([128, 1536], f32, tag="s")
                    nc.tensor.matmul(              # res chunk 6 (row-group lo)
                        ps[:, 0:512], kpiT[0:64, 384:512], qlo,
                        start=True, stop=True,
                    )
                    for hh in range(2):            # blk: kc0 -> bank1, kc1 -> bank2
                        g = 2 * c + hh
                        nc.tensor.matmul(
                            ps[:, 512 + hh * 256:512 + hh * 256 + 256],
                            ksT[0:64, g * 128:(g + 1) * 128],
                            qsT[0:64, g * 256:(g + 1) * 256],
                            start=(hh == 0), stop=(hh == 1),
                        )
                        nc.tensor.matmul(
                            ps[:, 1024 + hh * 256:1024 + hh * 256 + 256],
                            ksT[64:128, g * 128:(g + 1) * 128],
                            qsT[64:128, g * 256:(g + 1) * 256],
                            start=(hh == 0), stop=(hh == 1),
                        )
                    a = apool.tile([128, 1536], bf16, tag="a")
                    nc.scalar.activation(a[:], ps[:], EXP)
                    av.append(a)

                    # ---- att^T accumulation: residual then block ----
                    att_ps = attpsum.tile([D + 1, 512], f32, tag="att")
                    for j in range(NCH):
                        nc.tensor.matmul(
                            att_ps[:],
                            wpim[:, j, :],
                            av[j // 3][:, (j % 3) * 512:(j % 3) * 512 + 512],
                            start=(j == 0), stop=False,
                        )
                    for hh in range(2):
                        g = 2 * c + hh
                        nc.tensor.matmul(
                            att_ps[:, hh * 256:(hh + 1) * 256],
                            ws[:, 2 * g, :],
                            av[2][:, 512 + hh * 256:512 + hh * 256 + 256],
                            start=False, stop=False,
                        )
                        nc.tensor.matmul(
                            att_ps[:, hh * 256:(hh + 1) * 256],
                            ws[:, 2 * g + 1, :],
                            av[2][:, 1024 + hh * 256:1024 + hh * 256 + 256],
                            start=False, stop=(hh == 1),
                        )
                    att_sb = opool.tile([D + 1, 512], f32, tag="o")
                    nc.vector.tensor_copy(att_sb[:], att_ps[:])
                    nc.sync.dma_start(att_d[bh, :, c * 512:(c + 1) * 512], att_sb[:])
    nc.compile()
    return nc


def kernel(query, key, value, proj_dir, pm_init):
    global LAST_RESULT
    _ensure_ntff_hook()
    from concourse.bass_utils import run_bass_kernel_spmd

    query = np.asarray(query, F32)
    key = np.asarray(key, F32)
    value = np.asarray(value, F32)
    proj_dir = np.asarray(proj_dir, F32)
    pm_init = np.asarray(pm_init, F32)

    per_bh = _host_prep(query, key, value, proj_dir, pm_init)

    if "nc" not in _NC_CACHE:
        _NC_CACHE["nc"] = _build_nc()
    nc = _NC_CACHE["nc"]

    in_maps = []
    for core in range(NCORES):
        chunk = per_bh[core * BH_PER_CORE:(core + 1) * BH_PER_CORE]
        in_maps.append({
            "qsT": np.stack([d["qsT"] for d in chunk]),
            "ksT": np.stack([d["ksT"] for d in chunk]),
            "kpiT": np.stack([d["kpiT"] for d in chunk]),
            "ws": np.stack([d["ws"] for d in chunk]),
            "wpim": np.stack([d["wpim"] for d in chunk]),
        })

    trace = os.environ.get("CHA_TRACE", "0") == "1"
    res = run_bass_kernel_spmd(nc, in_maps, core_ids=list(range(NCORES)), trace=trace)
    LAST_RESULT = res

    est = np.empty((B, H, N, D), F32)
    for core in range(NCORES):
        att = res.results[core]["attT"]  # [BH_PER_CORE, 65, N]
        for j in range(BH_PER_CORE):
            bh = core * BH_PER_CORE + j
            b, h = bh // H, bh % H
            es = (att[j, :D, :] / att[j, D, :]).T      # [N, D] sorted order
            est[b, h, per_bh[bh]["q_idx"]] = es
    return est
